# revision 19
# baseline (speedup 1.0000x reference)
"""MoE GemmaMLP (top-2 of 8 experts + shared expert) on 8 trn2 NeuronCores.

v4: bf16, host-packed contiguous DMA, shared expert folded in as a
data-parallel group, zero-padding load balance with half-pair slots.

Sharding: expert-parallel with load balancing.  The host computes top-2
routing from router_logits and decomposes each expert's routed batches into
full pairs (2 batches, 512 moving columns) and at most one half pair
(1 batch, 256 moving columns).  These are packed into weight-stream groups
so that every core gets the identical (SPMD-uniform) slot config with no
padded dummy batches:
  - n22 two-pair expert groups + ns single-pair expert groups (full I),
  - one two-pair shared-expert group (data-parallel: core c owns batches
    4c..4c+3, full I),
  - leftover full/half pairs as tensor-parallel slots (I/8 slice per core,
    host-reduced across cores).

Everything the device touches is bf16 and packed on the host into the exact
SBUF tile layout, so every DMA is a plain contiguous [128, <=4096] copy
(128 descriptors x <=8KB) — minimizing HWDGE issue-path pressure, the
dominant HW overhead.  DMAs alternate between the SP and ACT HWDGE rings;
the first chunk's weights are interleaved with the x loads so the PE can
start after ~2 transfers.

Per group: expert weights stream from HBM exactly once, chunked 4 i-tiles
at a time; gate/up matmuls (stationary = weight tile, moving = x columns)
fill PSUM, gelu*up produces a^T in bf16, and the down projection contracts
each chunk with hg-interleaved PSUM chains (stationary a^T block loaded
once for both h-halves), accumulating into per-pair f32 SBUF tiles; the
last chunk's accumulation writes a bf16 staging tile directly, which is
DMA'd per pair as soon as it completes to a packed output blob the host
unpacks, weights (routing), and reduces.
"""

import numpy as np
from contextlib import ExitStack

import concourse.bass as bass
import concourse.mybir as mybir
import concourse.tile as tile
from concourse import bacc
from concourse.bass_utils import run_bass_kernel_spmd

B, S, H, I, E = 32, 256, 1024, 4096, 8
TOP_K = 2
NCORES = 8
P = 128
HT = H // P               # 8 h-tiles
S2 = 2 * S                # 512 moving columns (one full pair)
CHUNK = 4                 # i-tiles per weight-stream / down-accum chunk
NIT = I // P              # 32 i-tiles for a full expert

F32 = mybir.dt.float32
BF16 = mybir.dt.bfloat16
NPBF = mybir.dt.np(BF16)
GELU = mybir.ActivationFunctionType.Gelu_apprx_tanh


def _group(nc, pools, ring, inb, off, outb, sizes, ni, first=False):
    """One weight-stream group: pairs of `sizes` batches (2=full, 1=half)
    sharing one expert's weights over `ni` i-tiles."""
    npair = len(sizes)
    scols = [s * S for s in sizes]
    sumsc = sum(scols)
    pre = [sum(scols[:i]) for i in range(npair)]
    nch = ni // CHUNK
    CW = CHUNK * P        # 512 i-cols per chunk

    xt_t = [pools["xt"].tile([P, HT * S2], BF16, tag="xt", name=f"xt{pr}")
            for pr in range(npair)]
    out_sb = [pools["outsb"].tile([P, 8 * 512], F32, tag="outsb",
                                  name=f"osb{pr}")
              for pr in range(npair)] if nch > 1 else [None] * npair
    st_t = [pools["stage"].tile([P, 8 * 512], BF16, tag="stage",
                                name=f"st{pr}")
            for pr in range(npair)]

    # interleave x loads with chunk-0 weight loads so the PE can start
    # after ~2 parallel transfers; for the very first group split them in
    # halves so the PE starts after ~2 half transfers (dependencies are
    # tracked per byte range)
    wg_t = pools["wg"].tile([P, HT * CW], BF16, tag="wg")
    w0 = HT * scols[0]
    if first:
        ring()(xt_t[0][:, :w0 // 2], inb[off["xt"]][:, :w0 // 2])
        ring()(wg_t[:, :HT * CW // 2], inb[off["wg"]][:, :HT * CW // 2])
        ring()(xt_t[0][:, w0 // 2:w0], inb[off["xt"]][:, w0 // 2:w0])
        ring()(wg_t[:, HT * CW // 2:], inb[off["wg"]][:, HT * CW // 2:])
    else:
        ring()(xt_t[0][:, :w0], inb[off["xt"]][:, :w0])
        ring()(wg_t[:], inb[off["wg"]])
    for pr in range(1, npair):
        ring()(xt_t[pr][:, :HT * scols[pr]],
               inb[off["xt"] + pr][:, :HT * scols[pr]])
    wu_t = pools["wu"].tile([P, HT * CW], BF16, tag="wu")
    ring()(wu_t[:], inb[off["wu"]])
    wd_t = pools["wd"].tile([P, CHUNK * H], BF16, tag="wd")
    ring()(wd_t[:], inb[off["wd"]])

    for c in range(nch):
        if c > 0:
            wg_t = pools["wg"].tile([P, HT * CW], BF16, tag="wg")
            ring()(wg_t[:], inb[off["wg"] + c])
            wu_t = pools["wu"].tile([P, HT * CW], BF16, tag="wu")
            ring()(wu_t[:], inb[off["wu"] + c])
            wd_t = pools["wd"].tile([P, CHUNK * H], BF16, tag="wd")
            ring()(wd_t[:], inb[off["wd"] + c])
        at_t = pools["aT"].tile([P, CHUNK * 2 * S2], BF16, tag="aT")

        for it in range(CHUNK):
            ps_g = [pools["psgu"].tile([P, S2], F32, tag="ps",
                                       name=f"psg{pr}")
                    for pr in range(npair)]
            ps_u = [pools["psgu"].tile([P, S2], F32, tag="ps",
                                       name=f"psu{pr}")
                    for pr in range(npair)]
            for t in range(HT):
                col = t * CW + it * P
                for pr in range(npair):
                    nc.tensor.matmul(ps_g[pr][:, :scols[pr]],
                                     wg_t[:, col:col + P],
                                     xt_t[pr][:, t * scols[pr]:
                                               (t + 1) * scols[pr]],
                                     start=(t == 0), stop=(t == HT - 1))
            for t in range(HT):
                col = t * CW + it * P
                for pr in range(npair):
                    nc.tensor.matmul(ps_u[pr][:, :scols[pr]],
                                     wu_t[:, col:col + P],
                                     xt_t[pr][:, t * scols[pr]:
                                               (t + 1) * scols[pr]],
                                     start=(t == 0), stop=(t == HT - 1))
            for pr in range(npair):
                tmp = pools["tmp"].tile([P, S2], F32, tag="tmp")
                nc.scalar.activation(tmp[:, :scols[pr]],
                                     ps_g[pr][:, :scols[pr]], GELU)
                acol = it * sumsc + pre[pr]
                nc.vector.tensor_mul(at_t[:, acol:acol + scols[pr]],
                                     tmp[:, :scols[pr]],
                                     ps_u[pr][:, :scols[pr]])

        last = (c == nch - 1)
        for pr in range(npair):
            for ss in range(2 * sizes[pr]):
                sc = [pools["pssc"].tile([P, 512], F32, tag="sc",
                                         name=f"sc{hg}")
                      for hg in range(2)]
                for ci in range(CHUNK):
                    acol = ci * sumsc + pre[pr] + ss * P
                    for hg in range(2):
                        nc.tensor.matmul(
                            sc[hg][:], at_t[:, acol:acol + P],
                            wd_t[:, ci * H + hg * 512:ci * H + (hg + 1) * 512],
                            start=(ci == 0), stop=(ci == CHUNK - 1))
                for hg in range(2):
                    blk = slice((ss * 2 + hg) * 512, (ss * 2 + hg + 1) * 512)
                    if last:
                        # final chunk: write bf16 staging directly
                        if nch == 1:
                            nc.vector.tensor_copy(st_t[pr][:, blk], sc[hg][:])
                        else:
                            nc.vector.tensor_add(st_t[pr][:, blk],
                                                 out_sb[pr][:, blk], sc[hg][:])
                    elif c == 0:
                        nc.vector.tensor_copy(out_sb[pr][:, blk], sc[hg][:])
                    else:
                        nc.vector.tensor_add(out_sb[pr][:, blk],
                                             out_sb[pr][:, blk], sc[hg][:])
            if last:
                # split the store so the first half (written by earlier
                # ss blocks) streams out while the rest is still computed
                w = sizes[pr] * 4 * 512
                ring()(outb[off["out"] + pr][:, :w // 2],
                       st_t[pr][:, :w // 2])
                ring()(outb[off["out"] + pr][:, w // 2:w],
                       st_t[pr][:, w // 2:w])


def _layout(groups):
    """Row offsets of each group's tensors in the input/output blobs."""
    offs, r, orow = [], 0, 0
    for sizes, ni in groups:
        npair = len(sizes)
        nch = ni // CHUNK
        offs.append({"xt": r, "wg": r + npair, "wu": r + npair + nch,
                     "wd": r + npair + 2 * nch, "out": orow})
        r += npair + 3 * nch
        orow += npair
    return offs, r, orow


def _build_kernel(groups):
    """groups: tuple of (pair_sizes_tuple, n_itiles) per slot."""
    nc = bacc.Bacc("TRN2", target_bir_lowering=False, debug=False,
                   num_devices=NCORES)
    offs, nin, nout = _layout(groups)
    inb = nc.dram_tensor("inb", [nin, P, 4096], BF16,
                         kind="ExternalInput").ap()
    outb = nc.dram_tensor("outb", [nout, P, 4096], BF16,
                          kind="ExternalOutput").ap()

    with tile.TileContext(nc) as tc, ExitStack() as ctx:
        pools = {
            "xt": ctx.enter_context(tc.tile_pool(name="xt", bufs=4)),
            "psgu": ctx.enter_context(
                tc.tile_pool(name="psgu", bufs=5, space="PSUM")),
            "pssc": ctx.enter_context(
                tc.tile_pool(name="pssc", bufs=3, space="PSUM")),
            "tmp": ctx.enter_context(tc.tile_pool(name="tmp", bufs=3)),
            "aT": ctx.enter_context(tc.tile_pool(name="aT", bufs=2)),
            "outsb": ctx.enter_context(tc.tile_pool(name="outsb", bufs=3)),
            "stage": ctx.enter_context(tc.tile_pool(name="stage", bufs=3)),
            "wg": ctx.enter_context(tc.tile_pool(name="wg", bufs=2)),
            "wu": ctx.enter_context(tc.tile_pool(name="wu", bufs=2)),
            "wd": ctx.enter_context(tc.tile_pool(name="wd", bufs=2)),
        }
        rng = {"i": 0}

        def ring():
            eng = nc.sync if rng["i"] % 2 == 0 else nc.scalar
            rng["i"] += 1
            return eng.dma_start

        for gi, (sizes, ni) in enumerate(groups):
            _group(nc, pools, ring, inb, offs[gi], outb, sizes, ni,
                   first=(gi == 0))

    nc.compile()
    return nc


_KERNEL_CACHE = {}


def _get_kernel(groups):
    if groups not in _KERNEL_CACHE:
        _KERNEL_CACHE[groups] = _build_kernel(groups)
    return _KERNEL_CACHE[groups]


def _routing(router_logits):
    """Replicate reference routing in numpy f32: softmax, top-2, renorm."""
    rl = np.asarray(router_logits, np.float32)
    m = rl.max(axis=-1, keepdims=True)
    ex = np.exp(rl - m, dtype=np.float32)
    rw = ex / ex.sum(axis=-1, keepdims=True)
    sel = np.argsort(-rw, axis=-1, kind="stable")[:, :TOP_K]
    w = np.take_along_axis(rw, sel, axis=-1)
    w = w / w.sum(axis=-1, keepdims=True)
    return sel, w.astype(np.float32)


def _pack_gu(w):
    """[H, Ic] f32 -> [nch, 128, HT*CHUNK*128] bf16 (chunk, p, t, it, ii)."""
    Ic = w.shape[1]
    nch = Ic // (CHUNK * P)
    return np.ascontiguousarray(
        w.astype(NPBF).reshape(HT, P, nch, CHUNK * P)
        .transpose(2, 1, 0, 3).reshape(nch, P, HT * CHUNK * P))


def _pack_d(wd):
    """[Ir, H] f32 -> [nch, 128, CHUNK*H] bf16 (chunk, p, ci, h)."""
    Ir = wd.shape[0]
    nch = Ir // (CHUNK * P)
    return np.ascontiguousarray(
        wd.astype(NPBF).reshape(nch, CHUNK, P, H)
        .transpose(0, 2, 1, 3).reshape(nch, P, CHUNK * H))


def _pack_x(xb):
    """[nb, S, H] f32 -> [128, HT*nb*S] bf16 (p, t, b, s)."""
    nb = xb.shape[0]
    return np.ascontiguousarray(
        xb.astype(NPBF).reshape(nb, S, HT, P)
        .transpose(3, 2, 0, 1).reshape(P, HT * nb * S))


def _unpack_out(r, nb):
    """[128, nb*2048] bf16 -> [nb, S, H] f32."""
    return (r[:, :nb * 2048].astype(np.float32)
            .reshape(P, nb, 2, 2, 512)            # p, b, sblk, hg, hh
            .transpose(1, 2, 0, 3, 4)
            .reshape(nb, S, H))


def kernel(x, router_logits, skill_gate, skill_up, skill_down,
           shared_gate, shared_up, shared_down):
    x = np.asarray(x, np.float32)

    sel, w = _routing(router_logits)
    lists = [[] for _ in range(E)]
    wmap = np.zeros((B, E), np.float32)
    for b in range(B):
        for k in range(TOP_K):
            e = int(sel[b, k])
            lists[e].append(b)
            wmap[b, e] = w[b, k]

    # decompose each expert's routed batches into full pairs + <=1 half pair
    fulls, halves = [], []            # (e, (b0, b1)) / (e, (b0,))
    for e in range(E):
        bs = lists[e]
        for i in range(0, len(bs) - 1, 2):
            fulls.append((e, (bs[i], bs[i + 1])))
        if len(bs) % 2:
            halves.append((e, (bs[-1],)))

    # same-expert two-pair groups: floor(count/8) per core, uniform
    by_e = {}
    for f in fulls:
        by_e.setdefault(f[0], []).append(f)
    g22_all = []
    for e in sorted(by_e):
        fl = by_e[e]
        while len(fl) >= 2:
            g22_all.append((e, fl.pop()[1] + fl.pop()[1]))
    n22 = len(g22_all) // NCORES
    # dissolve unused 22-groups back into single pairs
    rest = [(e, ent[0:2]) for e, ent in g22_all[n22 * NCORES:]] + \
           [(e, ent[2:4]) for e, ent in g22_all[n22 * NCORES:]]
    g22 = g22_all[:n22 * NCORES]
    singles = [(e, f) for e, fl in sorted(by_e.items()) for _, f in fl] + rest
    ns = len(singles) // NCORES
    own1 = singles[:ns * NCORES]
    tp_full = singles[ns * NCORES:]
    tp_half = halves

    # slot list, identical shape sequence on every core.  kind is one of
    # "own" (per-core expert group), "shared", "tp" (replicated pair,
    # I/8 slice per core).  Half-pair tp slots trail to minimize the tail.
    slots = []
    for j in range(n22):
        slots.append(("own", None, None, "g22", j))
    for j in range(ns):
        slots.append(("own", None, None, "own1", j))
    slots.append(("shared", None, None))
    slots += [("tp", e, ent) for e, ent in tp_full + tp_half]

    cfg, kinds = [], []
    for s in slots:
        if s[0] == "tp":
            cfg.append(((len(s[2]),), CHUNK))
        else:
            src = s[3] if s[0] == "own" else None
            cfg.append(((2, 2) if (s[0] == "shared" or src == "g22")
                        else (2,), NIT))
    cfg = tuple(cfg)

    nc = _get_kernel(cfg)

    # pack weights once (bf16, SBUF layout)
    pg = [_pack_gu(np.asarray(skill_gate[e], np.float32)) for e in range(E)]
    pu = [_pack_gu(np.asarray(skill_up[e], np.float32)) for e in range(E)]
    pd = [_pack_d(np.asarray(skill_down[e], np.float32)) for e in range(E)]
    psg = _pack_gu(np.asarray(shared_gate, np.float32))
    psu = _pack_gu(np.asarray(shared_up, np.float32))
    psd = _pack_d(np.asarray(shared_down, np.float32))

    offs, nin, nout = _layout(cfg)
    in_maps = []
    core_ent = []       # per core, per slot: (e, batch tuple)
    for c in range(NCORES):
        blob = np.empty((nin, P, 4096), NPBF)
        ents = []
        for gi, s in enumerate(slots):
            o = offs[gi]
            if s[0] == "own":
                e, ent = (g22[c * n22 + s[4]] if s[3] == "g22"
                          else own1[c * ns + s[4]])
                wg_, wu_, wd_ = pg[e], pu[e], pd[e]
            elif s[0] == "shared":
                # data-parallel: core c owns batches 4c..4c+3, full I
                e, ent = None, tuple(range(4 * c, 4 * c + 4))
                wg_, wu_, wd_ = psg, psu, psd
            else:
                # tp slot: chunk c of expert e's packed weights is exactly
                # this core's i-slice [c*512, (c+1)*512)
                e, ent = s[1], tuple(s[2])
                wg_, wu_, wd_ = (pg[e][c:c + 1], pu[e][c:c + 1],
                                 pd[e][c:c + 1])
            ents.append((e, ent))
            pos = 0
            for pr, nb in enumerate(cfg[gi][0]):
                pk = _pack_x(x[list(ent[pos:pos + nb])])
                blob[o["xt"] + pr, :, :pk.shape[1]] = pk
                pos += nb
            nch = cfg[gi][1] // CHUNK
            blob[o["wg"]:o["wg"] + nch] = wg_
            blob[o["wu"]:o["wu"] + nch] = wu_
            blob[o["wd"]:o["wd"] + nch] = wd_
        core_ent.append(ents)
        in_maps.append({"inb": blob})

    res = run_bass_kernel_spmd(nc, in_maps, core_ids=list(range(NCORES)))
    kernel.last_exec_time_ns = res.exec_time_ns
    kernel.last_results = res
    kernel.last_nc = nc
    kernel.last_in_maps = in_maps

    out = np.zeros((B, S, H), np.float32)
    for gi, s in enumerate(slots):
        o = offs[gi]["out"]
        sizes = cfg[gi][0]
        if s[0] == "tp":
            # partial (I/8) sums — reduce across cores, then weight
            e, ent = s[1], tuple(s[2])
            nb = len(ent)
            acc = sum(_unpack_out(res.results[c]["outb"][o], nb)
                      for c in range(NCORES))
            for j in range(nb):
                out[ent[j]] += wmap[ent[j], e] * acc[j]
        else:
            for c in range(NCORES):
                e, ent = core_ent[c][gi]
                ob = res.results[c]["outb"]
                pos = 0
                for pr, nb in enumerate(sizes):
                    arr = _unpack_out(ob[o + pr], nb)
                    for j in range(nb):
                        b = ent[pos + j]
                        out[b] += (arr[j] if s[0] == "shared"
                                   else wmap[b, e] * arr[j])
                    pos += nb
    return out


# revision 26
# speedup vs baseline: 1.1135x; 1.1135x over previous
"""MoE GemmaMLP (top-2 of 8 experts + shared expert) on 8 trn2 NeuronCores.

v4: bf16, host-packed contiguous DMA, shared expert folded in as a
data-parallel group, zero-padding load balance with half-pair slots.

Sharding: expert-parallel with load balancing.  The host computes top-2
routing from router_logits and decomposes each expert's routed batches into
full pairs (2 batches, 512 moving columns) and at most one half pair
(1 batch, 256 moving columns).  These are packed into weight-stream groups
so that every core gets the identical (SPMD-uniform) slot config with no
padded dummy batches:
  - n22 two-pair expert groups + ns single-pair expert groups (full I),
  - one two-pair shared-expert group (data-parallel: core c owns batches
    4c..4c+3, full I),
  - leftover full/half pairs as tensor-parallel slots (I/8 slice per core,
    host-reduced across cores).

Everything the device touches is bf16 and packed on the host into the exact
SBUF tile layout, so every DMA is a plain contiguous [128, <=4096] copy
(128 descriptors x <=8KB) — minimizing HWDGE issue-path pressure, the
dominant HW overhead.  DMAs alternate between the SP and ACT HWDGE rings;
the first chunk's weights are interleaved with the x loads so the PE can
start after ~2 transfers.

Per group: expert weights stream from HBM exactly once, chunked 4 i-tiles
at a time; gate/up matmuls (stationary = weight tile, moving = x columns)
fill PSUM, gelu*up produces a^T in bf16, and the down projection contracts
each chunk with hg-interleaved PSUM chains (stationary a^T block loaded
once for both h-halves), accumulating into per-pair f32 SBUF tiles; the
last chunk's accumulation writes a bf16 staging tile directly, which is
DMA'd per pair as soon as it completes to a packed output blob the host
unpacks, weights (routing), and reduces.
"""

import numpy as np
from contextlib import ExitStack

import concourse.bass as bass
import concourse.mybir as mybir
import concourse.tile as tile
from concourse import bacc
from concourse.bass_utils import run_bass_kernel_spmd

B, S, H, I, E = 32, 256, 1024, 4096, 8
TOP_K = 2
NCORES = 8
P = 128
HT = H // P               # 8 h-tiles
S2 = 2 * S                # 512 moving columns (one full pair)
CHUNK = 4                 # i-tiles per weight-stream / down-accum chunk
NIT = I // P              # 32 i-tiles for a full expert

F32 = mybir.dt.float32
BF16 = mybir.dt.bfloat16
NPBF = mybir.dt.np(BF16)
GELU = mybir.ActivationFunctionType.Gelu_apprx_tanh
COPY = mybir.ActivationFunctionType.Copy


def _group(nc, pools, ring, inb, off, outb, sizes, ni, first=False):
    """One weight-stream group: pairs of `sizes` batches (2=full, 1=half)
    sharing one expert's weights over `ni` i-tiles."""
    npair = len(sizes)
    scols = [s * S for s in sizes]
    sumsc = sum(scols)
    pre = [sum(scols[:i]) for i in range(npair)]
    nch = ni // CHUNK
    CW = CHUNK * P        # 512 i-cols per chunk

    xt_t = [pools["xt"].tile([P, HT * S2], BF16, tag="xt", name=f"xt{pr}")
            for pr in range(npair)]
    out_sb = [pools["outsb"].tile([P, 8 * 512], F32, tag="outsb",
                                  name=f"osb{pr}")
              for pr in range(npair)] if nch > 1 else [None] * npair
    st_t = [pools["stage"].tile([P, 8 * 512], BF16, tag="stage",
                                name=f"st{pr}")
            for pr in range(npair)]

    # interleave x loads with chunk-0 weight loads so the PE can start
    # after ~2 parallel transfers; for the very first group split them in
    # halves ordered by first-use time (dependencies are tracked per byte
    # range): ring A gets xt pieces, ring B gets weight pieces
    wg_t = pools["wg"].tile([P, HT * CW], BF16, tag="wg")
    wu_t = pools["wu"].tile([P, HT * CW], BF16, tag="wu")
    wd_t = pools["wd"].tile([P, CHUNK * H], BF16, tag="wd")
    w0 = HT * scols[0]
    WH = HT * CW // 2
    if first and npair == 2:
        # pieces sized/ordered by first-use: wg's it0 prefix (0.25MiB,
        # contiguous in the it-major layout) + xt0's first half unblock
        # the PE; everything else streams in behind pair 0's first chains
        w1 = HT * scols[1]
        QW = HT * P
        ring()(xt_t[0][:, :w0 // 2], inb[off["xt"]][:, :w0 // 2])
        ring()(wg_t[:, :QW], inb[off["wg"]][:, :QW])
        ring()(xt_t[0][:, w0 // 2:w0], inb[off["xt"]][:, w0 // 2:w0])
        ring()(wg_t[:, QW:], inb[off["wg"]][:, QW:])
        ring()(xt_t[1][:, :w1 // 2], inb[off["xt"] + 1][:, :w1 // 2])
        ring()(wu_t[:, :WH], inb[off["wu"]][:, :WH])
        ring()(xt_t[1][:, w1 // 2:w1], inb[off["xt"] + 1][:, w1 // 2:w1])
        ring()(wu_t[:, WH:], inb[off["wu"]][:, WH:])
        ring()(wd_t[:], inb[off["wd"]])
    else:
        ring()(xt_t[0][:, :w0], inb[off["xt"]][:, :w0])
        ring()(wg_t[:], inb[off["wg"]])
        for pr in range(1, npair):
            ring()(xt_t[pr][:, :HT * scols[pr]],
                   inb[off["xt"] + pr][:, :HT * scols[pr]])
        ring()(wu_t[:], inb[off["wu"]])
        ring()(wd_t[:], inb[off["wd"]])

    for c in range(nch):
        if c > 0:
            wg_t = pools["wg"].tile([P, HT * CW], BF16, tag="wg")
            ring()(wg_t[:], inb[off["wg"] + c])
            wu_t = pools["wu"].tile([P, HT * CW], BF16, tag="wu")
            ring()(wu_t[:], inb[off["wu"] + c])
            wd_t = pools["wd"].tile([P, CHUNK * H], BF16, tag="wd")
            ring()(wd_t[:], inb[off["wd"] + c])
        at_t = pools["aT"].tile([P, CHUNK * 2 * S2], BF16, tag="aT")

        for it in range(CHUNK):
            ps_g = [pools["psgu"].tile([P, S2], F32, tag="ps",
                                       name=f"psg{pr}")
                    for pr in range(npair)]
            ps_u = [pools["psgu"].tile([P, S2], F32, tag="ps",
                                       name=f"psu{pr}")
                    for pr in range(npair)]
            if first and c == 0 and it == 0:
                # pr-major: pair 1's matmuls start a chain-length later,
                # hiding its x load behind pair 0's first chain
                for pr in range(npair):
                    for t in range(HT):
                        col = it * (HT * P) + t * P
                        nc.tensor.matmul(ps_g[pr][:, :scols[pr]],
                                         wg_t[:, col:col + P],
                                         xt_t[pr][:, t * scols[pr]:
                                                   (t + 1) * scols[pr]],
                                         start=(t == 0), stop=(t == HT - 1))
            else:
                for t in range(HT):
                    col = it * (HT * P) + t * P
                    for pr in range(npair):
                        nc.tensor.matmul(ps_g[pr][:, :scols[pr]],
                                         wg_t[:, col:col + P],
                                         xt_t[pr][:, t * scols[pr]:
                                                   (t + 1) * scols[pr]],
                                         start=(t == 0), stop=(t == HT - 1))
            for t in range(HT):
                col = it * (HT * P) + t * P
                for pr in range(npair):
                    nc.tensor.matmul(ps_u[pr][:, :scols[pr]],
                                     wu_t[:, col:col + P],
                                     xt_t[pr][:, t * scols[pr]:
                                               (t + 1) * scols[pr]],
                                     start=(t == 0), stop=(t == HT - 1))
            for pr in range(npair):
                tmp = pools["tmp"].tile([P, S2], F32, tag="tmp")
                nc.scalar.activation(tmp[:, :scols[pr]],
                                     ps_g[pr][:, :scols[pr]], GELU)
                acol = it * sumsc + pre[pr]
                nc.vector.tensor_mul(at_t[:, acol:acol + scols[pr]],
                                     tmp[:, :scols[pr]],
                                     ps_u[pr][:, :scols[pr]])

        last = (c == nch - 1)
        for pr in range(npair):
            for ss in range(2 * sizes[pr]):
                sc = [pools["pssc"].tile([P, 512], F32, tag="sc",
                                         name=f"sc{hg}")
                      for hg in range(2)]
                for ci in range(CHUNK):
                    acol = ci * sumsc + pre[pr] + ss * P
                    for hg in range(2):
                        nc.tensor.matmul(
                            sc[hg][:], at_t[:, acol:acol + P],
                            wd_t[:, ci * H + hg * 512:ci * H + (hg + 1) * 512],
                            start=(ci == 0), stop=(ci == CHUNK - 1))
                for hg in range(2):
                    blk = slice((ss * 2 + hg) * 512, (ss * 2 + hg + 1) * 512)
                    if last:
                        # final chunk: write bf16 staging directly; for
                        # single-chunk groups alternate DVE/ACT so the two
                        # h-half copies drain in parallel
                        if nch == 1:
                            if hg == 0:
                                nc.scalar.activation(st_t[pr][:, blk],
                                                     sc[hg][:], COPY)
                            else:
                                nc.vector.tensor_copy(st_t[pr][:, blk],
                                                      sc[hg][:])
                        else:
                            nc.vector.tensor_add(st_t[pr][:, blk],
                                                 out_sb[pr][:, blk], sc[hg][:])
                    elif c == 0:
                        nc.vector.tensor_copy(out_sb[pr][:, blk], sc[hg][:])
                    else:
                        nc.vector.tensor_add(out_sb[pr][:, blk],
                                             out_sb[pr][:, blk], sc[hg][:])
            if last:
                # split the store so the first half (written by earlier
                # ss blocks) streams out while the rest is still computed
                w = sizes[pr] * 4 * 512
                ring()(outb[off["out"] + pr][:, :w // 2],
                       st_t[pr][:, :w // 2])
                ring()(outb[off["out"] + pr][:, w // 2:w],
                       st_t[pr][:, w // 2:w])


def _layout(groups):
    """Row offsets of each group's tensors in the input/output blobs."""
    offs, r, orow = [], 0, 0
    for sizes, ni in groups:
        npair = len(sizes)
        nch = ni // CHUNK
        offs.append({"xt": r, "wg": r + npair, "wu": r + npair + nch,
                     "wd": r + npair + 2 * nch, "out": orow})
        r += npair + 3 * nch
        orow += npair
    return offs, r, orow


def _build_kernel(groups):
    """groups: tuple of (pair_sizes_tuple, n_itiles) per slot."""
    nc = bacc.Bacc("TRN2", target_bir_lowering=False, debug=False,
                   num_devices=NCORES)
    offs, nin, nout = _layout(groups)
    inb = nc.dram_tensor("inb", [nin, P, 4096], BF16,
                         kind="ExternalInput").ap()
    outb = nc.dram_tensor("outb", [nout, P, 4096], BF16,
                          kind="ExternalOutput").ap()

    with tile.TileContext(nc) as tc, ExitStack() as ctx:
        pools = {
            "xt": ctx.enter_context(tc.tile_pool(name="xt", bufs=4)),
            "psgu": ctx.enter_context(
                tc.tile_pool(name="psgu", bufs=5, space="PSUM")),
            "pssc": ctx.enter_context(
                tc.tile_pool(name="pssc", bufs=3, space="PSUM")),
            "tmp": ctx.enter_context(tc.tile_pool(name="tmp", bufs=3)),
            "aT": ctx.enter_context(tc.tile_pool(name="aT", bufs=2)),
            "outsb": ctx.enter_context(tc.tile_pool(name="outsb", bufs=3)),
            "stage": ctx.enter_context(tc.tile_pool(name="stage", bufs=3)),
            "wg": ctx.enter_context(tc.tile_pool(name="wg", bufs=2)),
            "wu": ctx.enter_context(tc.tile_pool(name="wu", bufs=2)),
            "wd": ctx.enter_context(tc.tile_pool(name="wd", bufs=2)),
        }
        rng = {"i": 0}

        def ring():
            eng = nc.sync if rng["i"] % 2 == 0 else nc.scalar
            rng["i"] += 1
            return eng.dma_start

        for gi, (sizes, ni) in enumerate(groups):
            _group(nc, pools, ring, inb, offs[gi], outb, sizes, ni,
                   first=(gi == 0))

    nc.compile()
    return nc


_KERNEL_CACHE = {}


def _get_kernel(groups):
    if groups not in _KERNEL_CACHE:
        _KERNEL_CACHE[groups] = _build_kernel(groups)
    return _KERNEL_CACHE[groups]


def _routing(router_logits):
    """Replicate reference routing in numpy f32: softmax, top-2, renorm."""
    rl = np.asarray(router_logits, np.float32)
    m = rl.max(axis=-1, keepdims=True)
    ex = np.exp(rl - m, dtype=np.float32)
    rw = ex / ex.sum(axis=-1, keepdims=True)
    sel = np.argsort(-rw, axis=-1, kind="stable")[:, :TOP_K]
    w = np.take_along_axis(rw, sel, axis=-1)
    w = w / w.sum(axis=-1, keepdims=True)
    return sel, w.astype(np.float32)


def _pack_gu(w):
    """[H, Ic] f32 -> [nch, 128, CHUNK*HT*128] bf16 (chunk, p, it, t, ii).

    it-major within the chunk so one i-tile's 8 stationary slices form a
    contiguous 0.25MiB prefix — lets the PE start on a quarter transfer.
    """
    Ic = w.shape[1]
    nch = Ic // (CHUNK * P)
    return np.ascontiguousarray(
        w.astype(NPBF).reshape(HT, P, nch, CHUNK, P)
        .transpose(2, 1, 3, 0, 4).reshape(nch, P, CHUNK * HT * P))


def _pack_d(wd):
    """[Ir, H] f32 -> [nch, 128, CHUNK*H] bf16 (chunk, p, ci, h)."""
    Ir = wd.shape[0]
    nch = Ir // (CHUNK * P)
    return np.ascontiguousarray(
        wd.astype(NPBF).reshape(nch, CHUNK, P, H)
        .transpose(0, 2, 1, 3).reshape(nch, P, CHUNK * H))


def _pack_x(xb):
    """[nb, S, H] f32 -> [128, HT*nb*S] bf16 (p, t, b, s)."""
    nb = xb.shape[0]
    return np.ascontiguousarray(
        xb.astype(NPBF).reshape(nb, S, HT, P)
        .transpose(3, 2, 0, 1).reshape(P, HT * nb * S))


def _unpack_out(r, nb):
    """[128, nb*2048] bf16 -> [nb, S, H] f32."""
    return (r[:, :nb * 2048].astype(np.float32)
            .reshape(P, nb, 2, 2, 512)            # p, b, sblk, hg, hh
            .transpose(1, 2, 0, 3, 4)
            .reshape(nb, S, H))


def kernel(x, router_logits, skill_gate, skill_up, skill_down,
           shared_gate, shared_up, shared_down):
    x = np.asarray(x, np.float32)

    sel, w = _routing(router_logits)
    lists = [[] for _ in range(E)]
    wmap = np.zeros((B, E), np.float32)
    for b in range(B):
        for k in range(TOP_K):
            e = int(sel[b, k])
            lists[e].append(b)
            wmap[b, e] = w[b, k]

    # decompose each expert's routed batches into full pairs + <=1 half pair
    fulls, halves = [], []            # (e, (b0, b1)) / (e, (b0,))
    for e in range(E):
        bs = lists[e]
        for i in range(0, len(bs) - 1, 2):
            fulls.append((e, (bs[i], bs[i + 1])))
        if len(bs) % 2:
            halves.append((e, (bs[-1],)))

    # same-expert two-pair groups: floor(count/8) per core, uniform
    by_e = {}
    for f in fulls:
        by_e.setdefault(f[0], []).append(f)
    g22_all = []
    for e in sorted(by_e):
        fl = by_e[e]
        while len(fl) >= 2:
            g22_all.append((e, fl.pop()[1] + fl.pop()[1]))
    n22 = len(g22_all) // NCORES
    # dissolve unused 22-groups back into single pairs
    rest = [(e, ent[0:2]) for e, ent in g22_all[n22 * NCORES:]] + \
           [(e, ent[2:4]) for e, ent in g22_all[n22 * NCORES:]]
    g22 = g22_all[:n22 * NCORES]
    singles = [(e, f) for e, fl in sorted(by_e.items()) for _, f in fl] + rest
    ns = len(singles) // NCORES
    own1 = singles[:ns * NCORES]
    tp_full = singles[ns * NCORES:]
    tp_half = halves

    # slot list, identical shape sequence on every core.  kind is one of
    # "own" (per-core expert group), "shared", "tp" (replicated pair,
    # I/8 slice per core).  Half-pair tp slots trail to minimize the tail.
    slots = []
    for j in range(n22):
        slots.append(("own", None, None, "g22", j))
    for j in range(ns):
        slots.append(("own", None, None, "own1", j))
    slots.append(("shared", None, None))
    slots += [("tp", e, ent) for e, ent in tp_full + tp_half]

    cfg, kinds = [], []
    for s in slots:
        if s[0] == "tp":
            cfg.append(((len(s[2]),), CHUNK))
        else:
            src = s[3] if s[0] == "own" else None
            cfg.append(((2, 2) if (s[0] == "shared" or src == "g22")
                        else (2,), NIT))
    cfg = tuple(cfg)

    nc = _get_kernel(cfg)

    # pack weights once (bf16, SBUF layout)
    pg = [_pack_gu(np.asarray(skill_gate[e], np.float32)) for e in range(E)]
    pu = [_pack_gu(np.asarray(skill_up[e], np.float32)) for e in range(E)]
    pd = [_pack_d(np.asarray(skill_down[e], np.float32)) for e in range(E)]
    psg = _pack_gu(np.asarray(shared_gate, np.float32))
    psu = _pack_gu(np.asarray(shared_up, np.float32))
    psd = _pack_d(np.asarray(shared_down, np.float32))

    offs, nin, nout = _layout(cfg)
    in_maps = []
    core_ent = []       # per core, per slot: (e, batch tuple)
    for c in range(NCORES):
        blob = np.empty((nin, P, 4096), NPBF)
        ents = []
        for gi, s in enumerate(slots):
            o = offs[gi]
            if s[0] == "own":
                e, ent = (g22[c * n22 + s[4]] if s[3] == "g22"
                          else own1[c * ns + s[4]])
                wg_, wu_, wd_ = pg[e], pu[e], pd[e]
            elif s[0] == "shared":
                # data-parallel: core c owns batches 4c..4c+3, full I
                e, ent = None, tuple(range(4 * c, 4 * c + 4))
                wg_, wu_, wd_ = psg, psu, psd
            else:
                # tp slot: chunk c of expert e's packed weights is exactly
                # this core's i-slice [c*512, (c+1)*512)
                e, ent = s[1], tuple(s[2])
                wg_, wu_, wd_ = (pg[e][c:c + 1], pu[e][c:c + 1],
                                 pd[e][c:c + 1])
            ents.append((e, ent))
            pos = 0
            for pr, nb in enumerate(cfg[gi][0]):
                pk = _pack_x(x[list(ent[pos:pos + nb])])
                blob[o["xt"] + pr, :, :pk.shape[1]] = pk
                pos += nb
            nch = cfg[gi][1] // CHUNK
            blob[o["wg"]:o["wg"] + nch] = wg_
            blob[o["wu"]:o["wu"] + nch] = wu_
            blob[o["wd"]:o["wd"] + nch] = wd_
        core_ent.append(ents)
        in_maps.append({"inb": blob})

    res = run_bass_kernel_spmd(nc, in_maps, core_ids=list(range(NCORES)))
    kernel.last_exec_time_ns = res.exec_time_ns
    kernel.last_results = res
    kernel.last_nc = nc
    kernel.last_in_maps = in_maps

    out = np.zeros((B, S, H), np.float32)
    for gi, s in enumerate(slots):
        o = offs[gi]["out"]
        sizes = cfg[gi][0]
        if s[0] == "tp":
            # partial (I/8) sums — reduce across cores, then weight
            e, ent = s[1], tuple(s[2])
            nb = len(ent)
            acc = sum(_unpack_out(res.results[c]["outb"][o], nb)
                      for c in range(NCORES))
            for j in range(nb):
                out[ent[j]] += wmap[ent[j], e] * acc[j]
        else:
            for c in range(NCORES):
                e, ent = core_ent[c][gi]
                ob = res.results[c]["outb"]
                pos = 0
                for pr, nb in enumerate(sizes):
                    arr = _unpack_out(ob[o + pr], nb)
                    for j in range(nb):
                        b = ent[pos + j]
                        out[b] += (arr[j] if s[0] == "shared"
                                   else wmap[b, e] * arr[j])
                    pos += nb
    return out


# revision 30
# speedup vs baseline: 1.2501x; 1.1227x over previous
"""MoE GemmaMLP (top-2 of 8 experts + shared expert) on 8 trn2 NeuronCores.

v4: bf16, host-packed contiguous DMA, shared expert folded in as a
data-parallel group, zero-padding load balance with half-pair slots.

Sharding: expert-parallel with load balancing.  The host computes top-2
routing from router_logits and decomposes each expert's routed batches into
full pairs (2 batches, 512 moving columns) and at most one half pair
(1 batch, 256 moving columns).  These are packed into weight-stream groups
so that every core gets the identical (SPMD-uniform) slot config with no
padded dummy batches:
  - n22 two-pair expert groups + ns single-pair expert groups (full I),
  - one two-pair shared-expert group (data-parallel: core c owns batches
    4c..4c+3, full I),
  - leftover full/half pairs as tensor-parallel slots (I/8 slice per core,
    host-reduced across cores).

Everything the device touches is bf16 and packed on the host into the exact
SBUF tile layout, so every DMA is a plain contiguous [128, <=4096] copy
(128 descriptors x <=8KB) — minimizing HWDGE issue-path pressure, the
dominant HW overhead.  DMAs alternate between the SP and ACT HWDGE rings;
the first chunk's weights are interleaved with the x loads so the PE can
start after ~2 transfers.

Per group: expert weights stream from HBM exactly once, chunked 4 i-tiles
at a time; gate/up matmuls (stationary = weight tile, moving = x columns)
fill PSUM, gelu*up produces a^T in bf16, and the down projection contracts
each chunk with hg-interleaved PSUM chains (stationary a^T block loaded
once for both h-halves), accumulating into per-pair f32 SBUF tiles; the
last chunk's accumulation writes a bf16 staging tile directly, which is
DMA'd per pair as soon as it completes to a packed output blob the host
unpacks, weights (routing), and reduces.
"""

import numpy as np
from contextlib import ExitStack

import concourse.bass as bass
import concourse.mybir as mybir
import concourse.tile as tile
from concourse import bacc
from concourse.bass_utils import run_bass_kernel_spmd

B, S, H, I, E = 32, 256, 1024, 4096, 8
TOP_K = 2
NCORES = 8
P = 128
HT = H // P               # 8 h-tiles
S2 = 2 * S                # 512 moving columns (one full pair)
CHUNK = 4                 # i-tiles per weight-stream / down-accum chunk
NIT = I // P              # 32 i-tiles for a full expert

F32 = mybir.dt.float32
BF16 = mybir.dt.bfloat16
NPBF = mybir.dt.np(BF16)
GELU = mybir.ActivationFunctionType.Gelu_apprx_tanh
COPY = mybir.ActivationFunctionType.Copy


def _group(nc, pools, ring, inb, off, outb, sizes, ni, first=False):
    """One weight-stream group: pairs of `sizes` batches (2=full, 1=half)
    sharing one expert's weights over `ni` i-tiles."""
    npair = len(sizes)
    scols = [s * S for s in sizes]
    sumsc = sum(scols)
    pre = [sum(scols[:i]) for i in range(npair)]
    nch = ni // CHUNK
    CW = CHUNK * P        # 512 i-cols per chunk

    xt_t = [pools["xt"].tile([P, HT * S2], BF16, tag="xt", name=f"xt{pr}")
            for pr in range(npair)]
    out_sb = [pools["outsb"].tile([P, 8 * 512], F32, tag="outsb",
                                  name=f"osb{pr}")
              for pr in range(npair)] if nch > 1 else [None] * npair
    st_t = [pools["stage"].tile([P, 8 * 512], BF16, tag="stage",
                                name=f"st{pr}")
            for pr in range(npair)]

    # interleave x loads with chunk-0 weight loads so the PE can start
    # after ~2 parallel transfers; for the very first group split them in
    # halves ordered by first-use time (dependencies are tracked per byte
    # range): ring A gets xt pieces, ring B gets weight pieces
    wg_t = pools["wg"].tile([P, HT * CW], BF16, tag="wg")
    wu_t = pools["wu"].tile([P, HT * CW], BF16, tag="wu")
    wd_t = pools["wd"].tile([P, CHUNK * H], BF16, tag="wd")
    w0 = HT * scols[0]
    WH = HT * CW // 2
    if first and npair == 2:
        # pieces sized/ordered by first-use: wg's it0 prefix (0.25MiB,
        # contiguous in the it-major layout) + xt0's first half unblock
        # the PE; everything else streams in behind pair 0's first chains
        w1 = HT * scols[1]
        QW = HT * P
        ring()(xt_t[0][:, :w0 // 2], inb[off["xt"]][:, :w0 // 2])
        ring()(wg_t[:, :QW], inb[off["wg"]][:, :QW])
        ring()(xt_t[0][:, w0 // 2:w0], inb[off["xt"]][:, w0 // 2:w0])
        ring()(wg_t[:, QW:], inb[off["wg"]][:, QW:])
        ring()(xt_t[1][:, :w1 // 2], inb[off["xt"] + 1][:, :w1 // 2])
        ring()(wu_t[:, :WH], inb[off["wu"]][:, :WH])
        ring()(xt_t[1][:, w1 // 2:w1], inb[off["xt"] + 1][:, w1 // 2:w1])
        ring()(wu_t[:, WH:], inb[off["wu"]][:, WH:])
        ring()(wd_t[:], inb[off["wd"]])
    else:
        ring()(xt_t[0][:, :w0], inb[off["xt"]][:, :w0])
        ring()(wg_t[:], inb[off["wg"]])
        for pr in range(1, npair):
            ring()(xt_t[pr][:, :HT * scols[pr]],
                   inb[off["xt"] + pr][:, :HT * scols[pr]])
        ring()(wu_t[:], inb[off["wu"]])
        ring()(wd_t[:], inb[off["wd"]])

    for c in range(nch):
        if c > 0:
            wg_t = pools["wg"].tile([P, HT * CW], BF16, tag="wg")
            ring()(wg_t[:], inb[off["wg"] + c])
            wu_t = pools["wu"].tile([P, HT * CW], BF16, tag="wu")
            ring()(wu_t[:], inb[off["wu"] + c])
            wd_t = pools["wd"].tile([P, CHUNK * H], BF16, tag="wd")
            ring()(wd_t[:], inb[off["wd"] + c])
        at_t = pools["aT"].tile([P, CHUNK * 2 * S2], BF16, tag="aT")

        for it in range(CHUNK):
            ps_g = [pools["psgu"].tile([P, S2], F32, tag="ps",
                                       name=f"psg{pr}")
                    for pr in range(npair)]
            ps_u = [pools["psgu"].tile([P, S2], F32, tag="ps",
                                       name=f"psu{pr}")
                    for pr in range(npair)]
            if first and c == 0 and it == 0:
                # pr-major: pair 1's matmuls start a chain-length later,
                # hiding its x load behind pair 0's first chain
                for pr in range(npair):
                    for t in range(HT):
                        col = it * (HT * P) + t * P
                        nc.tensor.matmul(ps_g[pr][:, :scols[pr]],
                                         wg_t[:, col:col + P],
                                         xt_t[pr][:, t * scols[pr]:
                                                   (t + 1) * scols[pr]],
                                         start=(t == 0), stop=(t == HT - 1))
            else:
                for t in range(HT):
                    col = it * (HT * P) + t * P
                    for pr in range(npair):
                        nc.tensor.matmul(ps_g[pr][:, :scols[pr]],
                                         wg_t[:, col:col + P],
                                         xt_t[pr][:, t * scols[pr]:
                                                   (t + 1) * scols[pr]],
                                         start=(t == 0), stop=(t == HT - 1))
            for t in range(HT):
                col = it * (HT * P) + t * P
                for pr in range(npair):
                    nc.tensor.matmul(ps_u[pr][:, :scols[pr]],
                                     wu_t[:, col:col + P],
                                     xt_t[pr][:, t * scols[pr]:
                                               (t + 1) * scols[pr]],
                                     start=(t == 0), stop=(t == HT - 1))
            for pr in range(npair):
                tmp = pools["tmp"].tile([P, S2], F32, tag="tmp")
                nc.scalar.activation(tmp[:, :scols[pr]],
                                     ps_g[pr][:, :scols[pr]], GELU)
                acol = it * sumsc + pre[pr]
                nc.vector.tensor_mul(at_t[:, acol:acol + scols[pr]],
                                     tmp[:, :scols[pr]],
                                     ps_u[pr][:, :scols[pr]])

        last = (c == nch - 1)
        for pr in range(npair):
            for ss in range(2 * sizes[pr]):
                sc = [pools["pssc"].tile([P, 512], F32, tag="sc",
                                         name=f"sc{hg}")
                      for hg in range(2)]
                for ci in range(CHUNK):
                    acol = ci * sumsc + pre[pr] + ss * P
                    for hg in range(2):
                        nc.tensor.matmul(
                            sc[hg][:], at_t[:, acol:acol + P],
                            wd_t[:, ci * H + hg * 512:ci * H + (hg + 1) * 512],
                            start=(ci == 0), stop=(ci == CHUNK - 1))
                for hg in range(2):
                    blk = slice((ss * 2 + hg) * 512, (ss * 2 + hg + 1) * 512)
                    if last:
                        # final chunk: write bf16 staging directly; for
                        # single-chunk groups alternate DVE/ACT so the two
                        # h-half copies drain in parallel
                        if nch == 1:
                            if hg == 0:
                                nc.scalar.activation(st_t[pr][:, blk],
                                                     sc[hg][:], COPY)
                            else:
                                nc.vector.tensor_copy(st_t[pr][:, blk],
                                                      sc[hg][:])
                        else:
                            nc.vector.tensor_add(st_t[pr][:, blk],
                                                 out_sb[pr][:, blk], sc[hg][:])
                    elif c == 0:
                        nc.vector.tensor_copy(out_sb[pr][:, blk], sc[hg][:])
                    else:
                        nc.vector.tensor_add(out_sb[pr][:, blk],
                                             out_sb[pr][:, blk], sc[hg][:])
            if last:
                # split the store so the first half (written by earlier
                # ss blocks) streams out while the rest is still computed
                w = sizes[pr] * 4 * 512
                ring()(outb[off["out"] + pr][:, :w // 2],
                       st_t[pr][:, :w // 2])
                ring()(outb[off["out"] + pr][:, w // 2:w],
                       st_t[pr][:, w // 2:w])


def _layout(groups):
    """Row offsets of each group's tensors in the input/output blobs."""
    offs, r, orow = [], 0, 0
    for sizes, ni in groups:
        npair = len(sizes)
        nch = ni // CHUNK
        offs.append({"xt": r, "wg": r + npair, "wu": r + npair + nch,
                     "wd": r + npair + 2 * nch, "out": orow})
        r += npair + 3 * nch
        orow += npair
    return offs, r, orow


def _build_kernel(groups):
    """groups: tuple of (pair_sizes_tuple, n_itiles) per slot."""
    nc = bacc.Bacc("TRN2", target_bir_lowering=False, debug=False,
                   num_devices=NCORES)
    offs, nin, nout = _layout(groups)
    inb = nc.dram_tensor("inb", [nin, P, 4096], BF16,
                         kind="ExternalInput").ap()
    outb = nc.dram_tensor("outb", [nout, P, 4096], BF16,
                          kind="ExternalOutput").ap()

    with tile.TileContext(nc) as tc, ExitStack() as ctx:
        pools = {
            "xt": ctx.enter_context(tc.tile_pool(name="xt", bufs=4)),
            "psgu": ctx.enter_context(
                tc.tile_pool(name="psgu", bufs=5, space="PSUM")),
            "pssc": ctx.enter_context(
                tc.tile_pool(name="pssc", bufs=3, space="PSUM")),
            "tmp": ctx.enter_context(tc.tile_pool(name="tmp", bufs=3)),
            "aT": ctx.enter_context(tc.tile_pool(name="aT", bufs=2)),
            "outsb": ctx.enter_context(tc.tile_pool(name="outsb", bufs=3)),
            "stage": ctx.enter_context(tc.tile_pool(name="stage", bufs=3)),
            "wg": ctx.enter_context(tc.tile_pool(name="wg", bufs=2)),
            "wu": ctx.enter_context(tc.tile_pool(name="wu", bufs=2)),
            "wd": ctx.enter_context(tc.tile_pool(name="wd", bufs=2)),
        }
        rng = {"i": 0}

        def ring():
            eng = nc.sync if rng["i"] % 2 == 0 else nc.scalar
            rng["i"] += 1
            return eng.dma_start

        for gi, (sizes, ni) in enumerate(groups):
            _group(nc, pools, ring, inb, offs[gi], outb, sizes, ni,
                   first=(gi == 0))

    nc.compile()
    return nc


_KERNEL_CACHE = {}


def _get_kernel(groups):
    if groups not in _KERNEL_CACHE:
        _KERNEL_CACHE[groups] = _build_kernel(groups)
    return _KERNEL_CACHE[groups]


def _routing(router_logits):
    """Replicate reference routing in numpy f32: softmax, top-2, renorm."""
    rl = np.asarray(router_logits, np.float32)
    m = rl.max(axis=-1, keepdims=True)
    ex = np.exp(rl - m, dtype=np.float32)
    rw = ex / ex.sum(axis=-1, keepdims=True)
    sel = np.argsort(-rw, axis=-1, kind="stable")[:, :TOP_K]
    w = np.take_along_axis(rw, sel, axis=-1)
    w = w / w.sum(axis=-1, keepdims=True)
    return sel, w.astype(np.float32)


def _pack_gu(w):
    """[H, Ic] f32 -> [nch, 128, CHUNK*HT*128] bf16 (chunk, p, it, t, ii).

    it-major within the chunk so one i-tile's 8 stationary slices form a
    contiguous 0.25MiB prefix — lets the PE start on a quarter transfer.
    """
    Ic = w.shape[1]
    nch = Ic // (CHUNK * P)
    return np.ascontiguousarray(
        w.astype(NPBF).reshape(HT, P, nch, CHUNK, P)
        .transpose(2, 1, 3, 0, 4).reshape(nch, P, CHUNK * HT * P))


def _pack_d(wd):
    """[Ir, H] f32 -> [nch, 128, CHUNK*H] bf16 (chunk, p, ci, h)."""
    Ir = wd.shape[0]
    nch = Ir // (CHUNK * P)
    return np.ascontiguousarray(
        wd.astype(NPBF).reshape(nch, CHUNK, P, H)
        .transpose(0, 2, 1, 3).reshape(nch, P, CHUNK * H))


def _pack_x(xb):
    """[nb, S, H] f32 -> [128, HT*nb*S] bf16 (p, t, b, s)."""
    nb = xb.shape[0]
    return np.ascontiguousarray(
        xb.astype(NPBF).reshape(nb, S, HT, P)
        .transpose(3, 2, 0, 1).reshape(P, HT * nb * S))


def _unpack_out(r, nb):
    """[128, nb*2048] bf16 -> [nb, S, H] f32."""
    return (r[:, :nb * 2048].astype(np.float32)
            .reshape(P, nb, 2, 2, 512)            # p, b, sblk, hg, hh
            .transpose(1, 2, 0, 3, 4)
            .reshape(nb, S, H))


def kernel(x, router_logits, skill_gate, skill_up, skill_down,
           shared_gate, shared_up, shared_down):
    x = np.asarray(x, np.float32)

    sel, w = _routing(router_logits)
    lists = [[] for _ in range(E)]
    wmap = np.zeros((B, E), np.float32)
    for b in range(B):
        for k in range(TOP_K):
            e = int(sel[b, k])
            lists[e].append(b)
            wmap[b, e] = w[b, k]

    # decompose each expert's routed batches into full pairs + <=1 half pair
    fulls, halves = [], []            # (e, (b0, b1)) / (e, (b0,))
    for e in range(E):
        bs = lists[e]
        for i in range(0, len(bs) - 1, 2):
            fulls.append((e, (bs[i], bs[i + 1])))
        if len(bs) % 2:
            halves.append((e, (bs[-1],)))

    # same-expert two-pair groups: floor(count/8) per core, uniform
    by_e = {}
    for f in fulls:
        by_e.setdefault(f[0], []).append(f)
    g22_all = []
    for e in sorted(by_e):
        fl = by_e[e]
        while len(fl) >= 2:
            g22_all.append((e, fl.pop()[1] + fl.pop()[1]))
    n22 = len(g22_all) // NCORES
    # dissolve unused 22-groups back into single pairs
    rest = [(e, ent[0:2]) for e, ent in g22_all[n22 * NCORES:]] + \
           [(e, ent[2:4]) for e, ent in g22_all[n22 * NCORES:]]
    g22 = g22_all[:n22 * NCORES]
    singles = [(e, f) for e, fl in sorted(by_e.items()) for _, f in fl] + rest
    ns = len(singles) // NCORES
    own1 = singles[:ns * NCORES]
    tp_full = singles[ns * NCORES:]
    tp_half = halves

    # slot list, identical shape sequence on every core.  kind is one of
    # "own" (per-core expert group), "shared", "tp" (replicated pair,
    # I/8 slice per core).  Half-pair tp slots trail to minimize the tail.
    slots = []
    for j in range(n22):
        slots.append(("own", None, None, "g22", j))
    for j in range(ns):
        slots.append(("own", None, None, "own1", j))
    slots.append(("shared", None, None))
    slots += [("tp", e, ent) for e, ent in tp_full + tp_half]

    cfg, kinds = [], []
    for s in slots:
        if s[0] == "tp":
            cfg.append(((len(s[2]),), CHUNK))
        else:
            src = s[3] if s[0] == "own" else None
            cfg.append(((2, 2) if (s[0] == "shared" or src == "g22")
                        else (2,), NIT))
    cfg = tuple(cfg)

    nc = _get_kernel(cfg)

    # pack weights once (bf16, SBUF layout)
    pg = [_pack_gu(np.asarray(skill_gate[e], np.float32)) for e in range(E)]
    pu = [_pack_gu(np.asarray(skill_up[e], np.float32)) for e in range(E)]
    pd = [_pack_d(np.asarray(skill_down[e], np.float32)) for e in range(E)]
    psg = _pack_gu(np.asarray(shared_gate, np.float32))
    psu = _pack_gu(np.asarray(shared_up, np.float32))
    psd = _pack_d(np.asarray(shared_down, np.float32))

    offs, nin, nout = _layout(cfg)
    in_maps = []
    core_ent = []       # per core, per slot: (e, batch tuple)
    for c in range(NCORES):
        blob = np.empty((nin, P, 4096), NPBF)
        ents = []
        for gi, s in enumerate(slots):
            o = offs[gi]
            if s[0] == "own":
                e, ent = (g22[c * n22 + s[4]] if s[3] == "g22"
                          else own1[c * ns + s[4]])
                wg_, wu_, wd_ = pg[e], pu[e], pd[e]
            elif s[0] == "shared":
                # data-parallel: core c owns batches 4c..4c+3, full I
                e, ent = None, tuple(range(4 * c, 4 * c + 4))
                wg_, wu_, wd_ = psg, psu, psd
            else:
                # tp slot: chunk c of expert e's packed weights is exactly
                # this core's i-slice [c*512, (c+1)*512)
                e, ent = s[1], tuple(s[2])
                wg_, wu_, wd_ = (pg[e][c:c + 1], pu[e][c:c + 1],
                                 pd[e][c:c + 1])
            ents.append((e, ent))
            pos = 0
            for pr, nb in enumerate(cfg[gi][0]):
                pk = _pack_x(x[list(ent[pos:pos + nb])])
                blob[o["xt"] + pr, :, :pk.shape[1]] = pk
                pos += nb
            nch = cfg[gi][1] // CHUNK
            blob[o["wg"]:o["wg"] + nch] = wg_
            blob[o["wu"]:o["wu"] + nch] = wu_
            blob[o["wd"]:o["wd"] + nch] = wd_
        core_ent.append(ents)
        in_maps.append({"inb": blob})

    res = run_bass_kernel_spmd(nc, in_maps, core_ids=list(range(NCORES)))
    kernel.last_exec_time_ns = res.exec_time_ns
    kernel.last_results = res
    kernel.last_nc = nc
    kernel.last_in_maps = in_maps

    out = np.zeros((B, S, H), np.float32)
    for gi, s in enumerate(slots):
        o = offs[gi]["out"]
        sizes = cfg[gi][0]
        if s[0] == "tp":
            # partial (I/8) sums — reduce across cores, then weight
            e, ent = s[1], tuple(s[2])
            nb = len(ent)
            acc = sum(_unpack_out(res.results[c]["outb"][o], nb)
                      for c in range(NCORES))
            for j in range(nb):
                out[ent[j]] += wmap[ent[j], e] * acc[j]
        else:
            for c in range(NCORES):
                e, ent = core_ent[c][gi]
                ob = res.results[c]["outb"]
                pos = 0
                for pr, nb in enumerate(sizes):
                    arr = _unpack_out(ob[o + pr], nb)
                    for j in range(nb):
                        b = ent[pos + j]
                        out[b] += (arr[j] if s[0] == "shared"
                                   else wmap[b, e] * arr[j])
                    pos += nb
    return out
